# revision 6
# baseline (speedup 1.0000x reference)
"""Bass/Trainium2 kernel for a 6-layer post-norm transformer LM (B=8192, T=8).

Data-parallel over batch across 8 NeuronCores; each core runs 1024 sequences
(8192 tokens). Activations are kept transposed ([C, tok], C on partitions) so
every projection is lhsT.T @ rhs in float32r (TF32-class, 1 cyc/row at N=512).
Attention (T=8, HS=64) uses block-diagonal packing on the PE array: scores via
a block-diag q lhsT plus a rank-65 mask matmul, softmax on ACT/DVE, then attT
is assembled into a block-diag rhs for the att@v matmul. LayerNorm statistics
via ones-vector matmuls; partition-broadcast of per-token stats via K=1
matmuls. The residual stream is streamed through DRAM between layers (f32r);
each layer's weights are resident in SBUF.
"""
import sys
sys.path.insert(0, '/opt/trn_rl_repo')

import numpy as np

B, T, V, C, H, HS, L, DFF = 8192, 8, 65, 512, 8, 64, 6, 2048
LN_EPS = 1e-5
NCORES = 8
SEQ = B // NCORES          # 1024 sequences per core
TOK = SEQ * T              # 8192 tokens per core
TN = 512                   # tokens per tile
NT = TOK // TN             # 16 tiles
BIG = 30000.0

_CACHE = {}


def _build():
    import concourse.bass as bass
    from concourse import bacc
    import concourse.mybir as mybir
    from concourse.tile import TileContext

    F32, F32R = mybir.dt.float32, mybir.dt.float32r
    AF = mybir.ActivationFunctionType
    ALU = mybir.AluOpType
    AX = mybir.AxisListType
    ds = bass.ds

    nc = bacc.Bacc("TRN2", target_bir_lowering=False, debug=False,
                   num_devices=NCORES)

    idxf = nc.dram_tensor("idxf", [TOK], F32, kind="ExternalInput")
    embp = nc.dram_tensor("embp", [128, C], F32, kind="ExternalInput")
    wqh = nc.dram_tensor("wqh", [L, C, C], F32, kind="ExternalInput")
    wkh = nc.dram_tensor("wkh", [L, C, C], F32, kind="ExternalInput")
    wvh = nc.dram_tensor("wvh", [L, C, C], F32, kind="ExternalInput")
    pwh = nc.dram_tensor("pwh", [L, C, C], F32, kind="ExternalInput")
    w1h = nc.dram_tensor("w1h", [L, C, DFF], F32, kind="ExternalInput")
    w2h = nc.dram_tensor("w2h", [L, DFF, C], F32, kind="ExternalInput")
    lmwh = nc.dram_tensor("lmwh", [C, 128], F32, kind="ExternalInput")
    vecsh = nc.dram_tensor("vecsh", [128, L * 40], F32, kind="ExternalInput")
    lmbh = nc.dram_tensor("lmbh", [128, 1], F32, kind="ExternalInput")
    iotav = nc.dram_tensor("iotav", [128, TN], F32, kind="ExternalInput")
    posoh = nc.dram_tensor("posoh", [8, TN], F32, kind="ExternalInput")
    maskL = nc.dram_tensor("maskL", [128, 128], F32, kind="ExternalInput")
    maskR = nc.dram_tensor("maskR", [128, 64], F32, kind="ExternalInput")
    identh = nc.dram_tensor("identh", [128, 128], F32, kind="ExternalInput")
    xres = nc.dram_tensor("xres", [4, 128, TOK], F32R)
    logT = nc.dram_tensor("logT", [V, TOK], F32, kind="ExternalOutput")

    PE = mybir.EngineType.PE

    with TileContext(nc) as tc:
        with tc.tile_pool(name="const", bufs=1) as cpool, \
             tc.tile_pool(name="wpool", bufs=1) as wpool, \
             tc.tile_pool(name="wstage", bufs=2) as wstage, \
             tc.tile_pool(name="act", bufs=1) as apool, \
             tc.tile_pool(name="attn", bufs=4) as atpool, \
             tc.tile_pool(name="ps_mm", bufs=3, space="PSUM") as ps_mm, \
             tc.tile_pool(name="ps_bc", bufs=2, space="PSUM") as ps_bc, \
             tc.tile_pool(name="ps_sm", bufs=3, space="PSUM") as ps_sm:

            def load_round_into(t, dram_ap, name, ko_count, m):
                # dram_ap: [128, ko_count, m]; stage+round in <=1024-col chunks
                for ko in range(ko_count):
                    for off in range(0, m, 1024):
                        w = min(1024, m - off)
                        st = wstage.tile([128, 1024], F32, tag="wstage",
                                         name=f"st_{name}_{ko}_{off}")
                        nc.sync.dma_start(st[:, :w],
                                          dram_ap[:, ko, off:off + w])
                        nc.vector.tensor_copy(t[:, ko, off:off + w],
                                              st[:, :w])
                return t

            def wload(dram_ap, ko_count, m, tag, name):
                t = wpool.tile([128, ko_count, m], F32R, tag=tag, name=name)
                return load_round_into(t, dram_ap, name, ko_count, m)

            # ---- persistent constants ----
            mL_s = wload(maskL.rearrange("p (a m) -> p a m", a=1),
                         1, 128, "mL", "mL_s")[:, 0]
            mR_s = wload(maskR.rearrange("p (a m) -> p a m", a=1),
                         1, 64, "mR", "mR_s")[:, 0]
            id_s = wload(identh.rearrange("p (a m) -> p a m", a=1),
                         1, 128, "id", "id_s")[:, 0]
            lmw_s = wload(lmwh.rearrange("(ko p) v -> p ko v", p=128),
                          4, 128, "lmw", "lmw_s")
            vecs_s = cpool.tile([128, L * 40], F32, name="vecs_s")
            nc.sync.dma_start(vecs_s[:], vecsh[:])
            lmb_s = cpool.tile([128, 1], F32, name="lmb_s")
            nc.sync.dma_start(lmb_s[:], lmbh[:])
            ones_c = cpool.tile([128, 1], F32R, name="ones_c")
            nc.vector.memset(ones_c[:].bitcast(F32), 1.0)
            ones_r = cpool.tile([1, 128], F32R, name="ones_r")
            nc.vector.memset(ones_r[:].bitcast(F32), 1.0)
            eps_t = cpool.tile([1, 1], F32, name="eps_t")
            nc.vector.memset(eps_t[:], LN_EPS)

            # block-diag scratch (off-diagonal zeros persist forever)
            qbd = cpool.tile([128, 8, 128], F32R, name="qbd")
            nc.vector.memset(qbd[:].bitcast(F32), 0.0)
            rhs_av = cpool.tile([128, 4, 8, 128], F32R, name="rhs_av")
            nc.vector.memset(rhs_av[:].bitcast(F32), 0.0)

            # ---- embedding (uses layer-phase slots; all free here) ----
            emb_s = apool.tile([128, 16, TN], F32R, tag="big1", name="emb_s")
            st_e = wstage.tile([128, 1024], F32, tag="wstage", name="st_e")
            nc.sync.dma_start(st_e[:, 0:C], embp[:])
            nc.vector.tensor_copy(emb_s[:, 0, 0:C], st_e[:, 0:C])
            iov_s = apool.tile([128, 4, TN], F32, tag="vt", name="iov_s")
            nc.sync.dma_start(iov_s[:, 0], iotav[:])
            oh = apool.tile([128, 4, TN], F32R, tag="x1c", name="oh")
            nc.vector.memset(oh[:, 0].bitcast(F32), 0.0)
            st_p = wstage.tile([128, 1024], F32, tag="wstage", name="st_p")
            nc.sync.dma_start(st_p[0:8, 0:TN], posoh[:])
            nc.vector.tensor_copy(oh[96:104, 0, :], st_p[0:8, 0:TN])

            for g in range(NT):
                idxb = apool.tile([128, 4, TN], F32, tag="kT", name="idxb")
                src = bass.AP(idxf, g * TN, [[0, 65], [1, TN]])
                nc.sync.dma_start(idxb[0:65, 0, :], src)
                nc.vector.tensor_tensor(oh[0:65, 0, :], iov_s[0:65, 0, :],
                                        idxb[0:65, 0, :], ALU.is_equal)
                xe = apool.tile([128, 4, TN], F32R, tag="xt", name="xe")
                for ch in range(4):
                    x_ps = ps_mm.tile([128, TN], F32, tag="mm", name="x_ps")
                    nc.tensor.matmul(x_ps[:],
                                     emb_s[:, 0, ch * 128:(ch + 1) * 128],
                                     oh[:, 0], start=True, stop=True)
                    nc.vector.tensor_copy(xe[:, ch], x_ps[:])
                    nc.sync.dma_start(xres[ch, :, g * TN:(g + 1) * TN],
                                      xe[:, ch])

            # ---- layers ----
            for l in range(L):
                wq_s = wload(wqh[l].rearrange("(ko p) m -> p ko m", p=128),
                             4, C, "wq", f"wq{l}")
                wk_s = wload(wkh[l].rearrange("(ko p) m -> p ko m", p=128),
                             4, C, "wk", f"wk{l}")
                wv_s = wload(wvh[l].rearrange("(ko p) m -> p ko m", p=128),
                             4, C, "wv", f"wv{l}")
                pw_s = wload(pwh[l].rearrange("(ko p) m -> p ko m", p=128),
                             4, C, "pw", f"pw{l}")
                w1_s = wload(w1h[l].rearrange("(ko p) m -> p ko m", p=128),
                             4, DFF, "w1", f"w1{l}")
                w2_s = wload(w2h[l].rearrange("(ko p) m -> p ko m", p=128),
                             16, C, "w2", f"w2{l}")

                def vcol(j, l=l):
                    return vecs_s[:, l * 40 + j:l * 40 + j + 1]

                def layer_norm(r, gcol, bcol, out_tag, vcol=vcol):
                    """r: [128,4,TN] f32r -> normalized y in slot out_tag"""
                    r2 = apool.tile([128, 4, TN], F32R, tag="vt", name="r2sq")
                    sum_ps = ps_sm.tile([1, TN], F32, tag="sm", name="sum_ps")
                    sq_ps = ps_sm.tile([1, TN], F32, tag="sm", name="sq_ps")
                    for ch in range(4):
                        nc.scalar.square(r2[:, ch], r[:, ch])
                    for ch in range(4):
                        nc.tensor.matmul(sum_ps[:], ones_c[:], r[:, ch],
                                         start=(ch == 0), stop=(ch == 3))
                    for ch in range(4):
                        nc.tensor.matmul(sq_ps[:], ones_c[:], r2[:, ch],
                                         start=(ch == 0), stop=(ch == 3))
                    musb = apool.tile([1, TN], F32R, tag="musb", name="musb")
                    nc.scalar.mul(musb[:], sum_ps[:], 1.0 / C)
                    mu2 = apool.tile([1, TN], F32, tag="mu2", name="mu2")
                    nc.scalar.square(mu2[:], musb[:])
                    varb = apool.tile([1, TN], F32, tag="varb", name="varb")
                    nc.vector.scalar_tensor_tensor(varb[:], sq_ps[:], 1.0 / C,
                                                   mu2[:], ALU.mult,
                                                   ALU.subtract)
                    sd = apool.tile([1, TN], F32, tag="sd", name="sd")
                    nc.scalar.activation(sd[:], varb[:], AF.Sqrt, bias=eps_t[:])
                    isd = apool.tile([1, TN], F32R, tag="isd", name="isd")
                    with nc.allow_low_precision(reason="f32r stats fine"):
                        nc.vector.reciprocal(isd[:], sd[:])
                    mu_bc = ps_bc.tile([128, TN], F32, tag="bc", name="mu_bc")
                    nc.tensor.matmul(mu_bc[:], ones_r[:], musb[:],
                                     start=True, stop=True)
                    is_bc = ps_bc.tile([128, TN], F32, tag="bc", name="is_bc")
                    nc.tensor.matmul(is_bc[:], ones_r[:], isd[:],
                                     start=True, stop=True)
                    y = apool.tile([128, 4, TN], F32R, tag=out_tag,
                                   name=f"y_{out_tag}")
                    for ch in range(4):
                        nc.vector.tensor_tensor(y[:, ch], r[:, ch], mu_bc[:],
                                                ALU.subtract)
                        nc.vector.scalar_tensor_tensor(y[:, ch], y[:, ch],
                                                       vcol(gcol + ch),
                                                       is_bc[:], ALU.mult,
                                                       ALU.mult)
                        nc.vector.tensor_scalar(y[:, ch], y[:, ch],
                                                vcol(bcol + ch), None, ALU.add)
                    return y

                with tc.For_i(0, NT, 1, hint_engines=(PE,)) as it:
                    toks = ds(it * TN, TN)
                    xt = apool.tile([128, 4, TN], F32R, tag="xt", name="xt")
                    for ch in range(4):
                        nc.sync.dma_start(xt[:, ch], xres[ch, :, toks])

                    # QKV
                    qT = apool.tile([128, 4, TN], F32R, tag="x1c", name="qT")
                    kT = apool.tile([128, 4, TN], F32R, tag="kT", name="kT")
                    vt = apool.tile([128, 4, C], F32R, tag="vt", name="vt")
                    for ch in range(4):
                        q_ps = ps_mm.tile([128, TN], F32, tag="mm", name="q_ps")
                        for ko in range(4):
                            nc.tensor.matmul(
                                q_ps[:], wq_s[:, ko, ch * 128:(ch + 1) * 128],
                                xt[:, ko], start=(ko == 0), stop=(ko == 3))
                        nc.scalar.copy(qT[:, ch], q_ps[:])
                        k_ps = ps_mm.tile([128, TN], F32, tag="mm", name="k_ps")
                        for ko in range(4):
                            nc.tensor.matmul(
                                k_ps[:], wk_s[:, ko, ch * 128:(ch + 1) * 128],
                                xt[:, ko], start=(ko == 0), stop=(ko == 3))
                        nc.scalar.copy(kT[:, ch], k_ps[:])
                        v_ps = ps_mm.tile([128, C], F32, tag="mm", name="v_ps")
                        for ko in range(4):
                            nc.tensor.matmul(
                                v_ps[:], xt[:, ko, ch * 128:(ch + 1) * 128],
                                wv_s[:, ko, :], start=(ko == 0), stop=(ko == 3))
                        nc.vector.tensor_copy(vt[:, ch], v_ps[:])

                    # attention: hp-outer score/softmax, then all att@v
                    for hp in range(4):
                        qsrc = qT[:, hp].rearrange("p (g b t) -> p g b t",
                                                   g=8, b=8, t=8)
                        qdst = qbd[:].rearrange("p g (b h t) -> p g b h t",
                                                b=8, h=2, t=8)
                        nc.vector.tensor_copy(qdst[0:64, :, :, 0, :],
                                              qsrc[0:64])
                        nc.vector.tensor_copy(qdst[64:128, :, :, 1, :],
                                              qsrc[64:128])
                        for g8 in range(8):
                            s_ps = ps_sm.tile([128, 64], F32, tag="sm",
                                              name="s_ps")
                            nc.tensor.matmul(
                                s_ps[:], qbd[:, g8, :],
                                kT[:, hp, g8 * 64:(g8 + 1) * 64],
                                start=True, stop=False)
                            nc.tensor.matmul(s_ps[:], mL_s[:], mR_s[:],
                                             start=False, stop=True)
                            atte = atpool.tile([128, 64], F32R, tag="atte",
                                               name="atte")
                            nc.scalar.activation(atte[:], s_ps[:], AF.Exp)
                            zt = atpool.tile([128, 1], F32, tag="zt",
                                             name="zt")
                            nc.vector.tensor_reduce(zt[:], atte[:], AX.X,
                                                    ALU.add)
                            rt = atpool.tile([128, 1], F32, tag="rt",
                                             name="rt")
                            nc.vector.reciprocal(rt[:], zt[:])
                            nc.vector.tensor_scalar_mul(atte[:], atte[:],
                                                        rt[:])
                            at_ps = ps_sm.tile([64, 128], F32R, tag="sm",
                                               name="at_ps")
                            nc.tensor.transpose(at_ps[:], atte[:], id_s[:])
                            d = rhs_av[:, g8 // 2].rearrange(
                                "p h (gg b t) -> p h gg b t",
                                gg=2, b=8, t=8)[(g8 % 2) * 64:
                                                (g8 % 2) * 64 + 64,
                                                hp * 2:(hp + 1) * 2,
                                                g8 % 2, :, :]
                            s2 = at_ps[:].rearrange("p (b h t) -> p h b t",
                                                    b=8, h=2, t=8)
                            nc.vector.tensor_copy(d, s2)
                    ot = apool.tile([128, 4, TN], F32R, tag="x1c", name="ot")
                    for g16 in range(4):
                        for h in range(8):
                            o_ps = ps_sm.tile([64, 128], F32, tag="sm",
                                              name="o_ps")
                            nc.tensor.matmul(
                                o_ps[:], vt[:, g16, h * 64:(h + 1) * 64],
                                rhs_av[:, g16, h, :], start=True, stop=True)
                            nc.scalar.copy(
                                ot[(h % 2) * 64:(h % 2) * 64 + 64, h // 2,
                                   g16 * 128:(g16 + 1) * 128], o_ps[:])

                    # proj + residual
                    r1 = apool.tile([128, 16, TN], F32R, tag="big1",
                                    name="r1")
                    for ch in range(4):
                        p_ps = ps_mm.tile([128, TN], F32, tag="mm", name="p_ps")
                        for ko in range(4):
                            nc.tensor.matmul(
                                p_ps[:], pw_s[:, ko, ch * 128:(ch + 1) * 128],
                                ot[:, ko], start=(ko == 0), stop=(ko == 3))
                        nc.vector.scalar_tensor_tensor(
                            r1[:, ch], p_ps[:], vcol(16 + ch), xt[:, ch],
                            ALU.add, ALU.add)
                    x1 = layer_norm(r1[:, 0:4], 0, 4, "x1c")

                    # FFN
                    hrl = apool.tile([128, 16, TN], F32R, tag="big1",
                                     name="hrl")
                    for dch in range(16):
                        h_ps = ps_mm.tile([128, TN], F32, tag="mm", name="h_ps")
                        for ko in range(4):
                            nc.tensor.matmul(
                                h_ps[:], w1_s[:, ko, dch * 128:(dch + 1) * 128],
                                x1[:, ko], start=(ko == 0), stop=(ko == 3))
                        nc.scalar.activation(hrl[:, dch], h_ps[:], AF.Relu,
                                             bias=vcol(24 + dch))
                    r2n = apool.tile([128, 4, TN], F32R, tag="xt", name="r2n")
                    for ch in range(4):
                        f_ps = ps_mm.tile([128, TN], F32, tag="mm", name="f_ps")
                        for ko in range(16):
                            nc.tensor.matmul(
                                f_ps[:], w2_s[:, ko, ch * 128:(ch + 1) * 128],
                                hrl[:, ko], start=(ko == 0), stop=(ko == 15))
                        nc.vector.scalar_tensor_tensor(
                            r2n[:, ch], f_ps[:], vcol(20 + ch), x1[:, ch],
                            ALU.add, ALU.add)
                    x2 = layer_norm(r2n, 8, 12, "kT")

                    if l < L - 1:
                        for ch in range(4):
                            nc.sync.dma_start(xres[ch, :, toks], x2[:, ch])
                    else:
                        lg_ps = ps_mm.tile([128, TN], F32, tag="mm",
                                           name="lg_ps")
                        for ko in range(4):
                            nc.tensor.matmul(lg_ps[:], lmw_s[:, ko, :],
                                             x2[:, ko],
                                             start=(ko == 0), stop=(ko == 3))
                        lgs = apool.tile([128, 4, TN], F32, tag="vt",
                                         name="lgs")
                        nc.vector.tensor_scalar(lgs[0:V, 0, :], lg_ps[0:V, :],
                                                lmb_s[0:V, :], None, ALU.add)
                        nc.sync.dma_start(logT[:, toks], lgs[0:V, 0, :])
    nc.compile()
    return nc


def _prep_inputs(idx, tok_table, pos_table, wq, wk, wv, proj_w, proj_b,
                 w1, b1, w2, b2, ln1_g, ln1_b, ln2_g, ln2_b, lm_w, lm_b):
    f = np.float32
    scale = C ** (-0.5)
    wqh = np.ascontiguousarray(
        wq.transpose(0, 2, 1, 3).reshape(L, C, C) * scale).astype(f)
    wkh = np.ascontiguousarray(
        wk.transpose(0, 2, 1, 3).reshape(L, C, C)).astype(f)
    wvh = np.ascontiguousarray(
        wv.transpose(0, 2, 1, 3).reshape(L, C, C)).astype(f)
    pwh = np.ascontiguousarray(proj_w).astype(f)
    w1h = np.ascontiguousarray(w1).astype(f)
    w2h = np.ascontiguousarray(w2).astype(f)
    lmwh = np.zeros((C, 128), f)
    lmwh[:, :V] = lm_w
    embp = np.zeros((128, C), f)
    embp[:V] = tok_table
    embp[96:96 + T] = pos_table
    vecsh = np.zeros((128, L * 40), f)
    for l in range(L):
        for ch in range(4):
            sl = slice(ch * 128, (ch + 1) * 128)
            vecsh[:, l * 40 + 0 + ch] = ln1_g[l, sl]
            vecsh[:, l * 40 + 4 + ch] = ln1_b[l, sl]
            vecsh[:, l * 40 + 8 + ch] = ln2_g[l, sl]
            vecsh[:, l * 40 + 12 + ch] = ln2_b[l, sl]
            vecsh[:, l * 40 + 16 + ch] = proj_b[l, sl]
            vecsh[:, l * 40 + 20 + ch] = b2[l, sl]
        for dch in range(16):
            vecsh[:, l * 40 + 24 + dch] = b1[l, dch * 128:(dch + 1) * 128]
    lmbh = np.zeros((128, 1), f)
    lmbh[:V, 0] = lm_b
    iotav = np.broadcast_to(
        np.arange(128, dtype=f)[:, None], (128, TN)).copy()
    posoh = np.zeros((8, TN), f)
    for c in range(TN):
        posoh[c % 8, c] = 1.0
    mLm = np.zeros((128, 128), f)
    mRm = np.zeros((128, 64), f)
    mLm[0, :] = -BIG
    mRm[0, :] = 1.0
    for j in range(8):
        for sp in range(8):
            kk = 1 + j * 8 + sp
            for t in range(8):
                if t >= sp:
                    for h in range(2):
                        mLm[kk, j * 16 + h * 8 + t] = BIG
            mRm[kk, j * 8 + sp] = 1.0
    ident = np.eye(128, dtype=f)
    shared = dict(embp=embp, wqh=wqh, wkh=wkh, wvh=wvh, pwh=pwh, w1h=w1h,
                  w2h=w2h, lmwh=lmwh, vecsh=vecsh, lmbh=lmbh, iotav=iotav,
                  posoh=posoh, maskL=mLm, maskR=mRm, identh=ident)
    in_maps = []
    for c in range(NCORES):
        m = dict(shared)
        m["idxf"] = np.ascontiguousarray(
            idx[c * SEQ:(c + 1) * SEQ].reshape(-1)).astype(f)
        in_maps.append(m)
    return in_maps


def kernel(**inputs):
    from concourse.bass_utils import run_bass_kernel_spmd
    if "nc" not in _CACHE:
        _CACHE["nc"] = _build()
    nc = _CACHE["nc"]
    in_maps = _prep_inputs(**{k: np.asarray(v) for k, v in inputs.items()})
    res = run_bass_kernel_spmd(nc, in_maps, list(range(NCORES)))
    out = np.empty((B, T, V), np.float32)
    for c in range(NCORES):
        lt = res.results[c]["logT"]           # [V, TOK]
        out[c * SEQ:(c + 1) * SEQ] = (
            lt.reshape(V, SEQ, T).transpose(1, 2, 0))
    return out


if __name__ == "__main__":
    _build()
    print("built ok")


# revision 7
# speedup vs baseline: 1.0384x; 1.0384x over previous
"""Bass/Trainium2 kernel for a 6-layer post-norm transformer LM (B=8192, T=8).

Data-parallel over batch across 8 NeuronCores; each core runs 1024 sequences
(8192 tokens). Activations are kept transposed ([C, tok], C on partitions) so
every projection is lhsT.T @ rhs in float32r (TF32-class, 1 cyc/row at N=512).
Attention (T=8, HS=64) uses block-diagonal packing on the PE array: scores via
a block-diag q lhsT plus a rank-65 mask matmul, softmax on ACT/DVE, then attT
is assembled into a block-diag rhs for the att@v matmul. LayerNorm statistics
via ones-vector matmuls; partition-broadcast of per-token stats via K=1
matmuls. The residual stream is streamed through DRAM between layers (f32r);
each layer's weights are resident in SBUF.
"""
import sys
sys.path.insert(0, '/opt/trn_rl_repo')

import numpy as np

B, T, V, C, H, HS, L, DFF = 8192, 8, 65, 512, 8, 64, 6, 2048
LN_EPS = 1e-5
NCORES = 8
SEQ = B // NCORES          # 1024 sequences per core
TOK = SEQ * T              # 8192 tokens per core
TN = 512                   # tokens per tile
NT = TOK // TN             # 16 tiles
BIG = 30000.0
STATIC = True

_CACHE = {}


def _build():
    import concourse.bass as bass
    from concourse import bacc
    import concourse.mybir as mybir
    from concourse.tile import TileContext

    F32, F32R = mybir.dt.float32, mybir.dt.float32r
    AF = mybir.ActivationFunctionType
    ALU = mybir.AluOpType
    AX = mybir.AxisListType
    ds = bass.ds

    nc = bacc.Bacc("TRN2", target_bir_lowering=False, debug=False,
                   num_devices=NCORES)

    idxf = nc.dram_tensor("idxf", [TOK], F32, kind="ExternalInput")
    embp = nc.dram_tensor("embp", [128, C], F32, kind="ExternalInput")
    wqh = nc.dram_tensor("wqh", [L, C, C], F32, kind="ExternalInput")
    wkh = nc.dram_tensor("wkh", [L, C, C], F32, kind="ExternalInput")
    wvh = nc.dram_tensor("wvh", [L, C, C], F32, kind="ExternalInput")
    pwh = nc.dram_tensor("pwh", [L, C, C], F32, kind="ExternalInput")
    w1h = nc.dram_tensor("w1h", [L, C, DFF], F32, kind="ExternalInput")
    w2h = nc.dram_tensor("w2h", [L, DFF, C], F32, kind="ExternalInput")
    lmwh = nc.dram_tensor("lmwh", [C, 128], F32, kind="ExternalInput")
    vecsh = nc.dram_tensor("vecsh", [128, L * 40], F32, kind="ExternalInput")
    lmbh = nc.dram_tensor("lmbh", [128, 1], F32, kind="ExternalInput")
    iotav = nc.dram_tensor("iotav", [128, TN], F32, kind="ExternalInput")
    posoh = nc.dram_tensor("posoh", [8, TN], F32, kind="ExternalInput")
    maskL = nc.dram_tensor("maskL", [128, 128], F32, kind="ExternalInput")
    maskR = nc.dram_tensor("maskR", [128, 64], F32, kind="ExternalInput")
    identh = nc.dram_tensor("identh", [128, 128], F32, kind="ExternalInput")
    xres = nc.dram_tensor("xres", [4, 128, TOK], F32R)
    logT = nc.dram_tensor("logT", [V, TOK], F32, kind="ExternalOutput")

    PE = mybir.EngineType.PE

    with TileContext(nc) as tc:
        with tc.tile_pool(name="const", bufs=1) as cpool, \
             tc.tile_pool(name="wpool", bufs=1) as wpool, \
             tc.tile_pool(name="wstage", bufs=2) as wstage, \
             tc.tile_pool(name="act", bufs=1) as apool, \
             tc.tile_pool(name="attn", bufs=4) as atpool, \
             tc.tile_pool(name="ps_mm", bufs=3, space="PSUM") as ps_mm, \
             tc.tile_pool(name="ps_bc", bufs=2, space="PSUM") as ps_bc, \
             tc.tile_pool(name="ps_sm", bufs=3, space="PSUM") as ps_sm:

            def load_round_into(t, dram_ap, name, ko_count, m):
                # dram_ap: [128, ko_count, m]; stage+round in <=1024-col chunks
                for ko in range(ko_count):
                    for off in range(0, m, 1024):
                        w = min(1024, m - off)
                        st = wstage.tile([128, 1024], F32, tag="wstage",
                                         name=f"st_{name}_{ko}_{off}")
                        nc.sync.dma_start(st[:, :w],
                                          dram_ap[:, ko, off:off + w])
                        nc.vector.tensor_copy(t[:, ko, off:off + w],
                                              st[:, :w])
                return t

            def wload(dram_ap, ko_count, m, tag, name):
                t = wpool.tile([128, ko_count, m], F32R, tag=tag, name=name)
                return load_round_into(t, dram_ap, name, ko_count, m)

            # ---- persistent constants ----
            mL_s = wload(maskL.rearrange("p (a m) -> p a m", a=1),
                         1, 128, "mL", "mL_s")[:, 0]
            mR_s = wload(maskR.rearrange("p (a m) -> p a m", a=1),
                         1, 64, "mR", "mR_s")[:, 0]
            id_s = wload(identh.rearrange("p (a m) -> p a m", a=1),
                         1, 128, "id", "id_s")[:, 0]
            lmw_s = wload(lmwh.rearrange("(ko p) v -> p ko v", p=128),
                          4, 128, "lmw", "lmw_s")
            vecs_s = cpool.tile([128, L * 40], F32, name="vecs_s")
            nc.sync.dma_start(vecs_s[:], vecsh[:])
            lmb_s = cpool.tile([128, 1], F32, name="lmb_s")
            nc.sync.dma_start(lmb_s[:], lmbh[:])
            ones_c = cpool.tile([128, 1], F32R, name="ones_c")
            nc.vector.memset(ones_c[:].bitcast(F32), 1.0)
            ones_r = cpool.tile([1, 128], F32R, name="ones_r")
            nc.vector.memset(ones_r[:].bitcast(F32), 1.0)
            eps_t = cpool.tile([1, 1], F32, name="eps_t")
            nc.vector.memset(eps_t[:], LN_EPS)

            # block-diag scratch (off-diagonal zeros persist forever)
            qbd = cpool.tile([128, 8, 128], F32R, name="qbd")
            nc.vector.memset(qbd[:].bitcast(F32), 0.0)
            rhs_av = cpool.tile([128, 4, 8, 128], F32R, name="rhs_av")
            nc.vector.memset(rhs_av[:].bitcast(F32), 0.0)

            # ---- embedding (uses layer-phase slots; all free here) ----
            emb_s = apool.tile([128, 16, TN], F32R, tag="big1", name="emb_s")
            st_e = wstage.tile([128, 1024], F32, tag="wstage", name="st_e")
            nc.sync.dma_start(st_e[:, 0:C], embp[:])
            nc.vector.tensor_copy(emb_s[:, 0, 0:C], st_e[:, 0:C])
            iov_s = apool.tile([128, 4, TN], F32, tag="vt", name="iov_s")
            nc.sync.dma_start(iov_s[:, 0], iotav[:])
            oh = apool.tile([128, 4, TN], F32R, tag="x1c", name="oh")
            nc.vector.memset(oh[:, 0].bitcast(F32), 0.0)
            st_p = wstage.tile([128, 1024], F32, tag="wstage", name="st_p")
            nc.sync.dma_start(st_p[0:8, 0:TN], posoh[:])
            nc.vector.tensor_copy(oh[96:104, 0, :], st_p[0:8, 0:TN])

            for g in range(NT):
                idxb = apool.tile([128, 4, TN], F32, tag="kT", name="idxb")
                src = bass.AP(idxf, g * TN, [[0, 65], [1, TN]])
                nc.sync.dma_start(idxb[0:65, 0, :], src)
                nc.vector.tensor_tensor(oh[0:65, 0, :], iov_s[0:65, 0, :],
                                        idxb[0:65, 0, :], ALU.is_equal)
                xe = apool.tile([128, 4, TN], F32R, tag="xt", name="xe")
                for ch in range(4):
                    x_ps = ps_mm.tile([128, TN], F32, tag="mm", name="x_ps")
                    nc.tensor.matmul(x_ps[:],
                                     emb_s[:, 0, ch * 128:(ch + 1) * 128],
                                     oh[:, 0], start=True, stop=True)
                    nc.vector.tensor_copy(xe[:, ch], x_ps[:])
                    nc.sync.dma_start(xres[ch, :, g * TN:(g + 1) * TN],
                                      xe[:, ch])

            # ---- layers ----
            for l in range(L):
                wq_s = wload(wqh[l].rearrange("(ko p) m -> p ko m", p=128),
                             4, C, "wq", f"wq{l}")
                wk_s = wload(wkh[l].rearrange("(ko p) m -> p ko m", p=128),
                             4, C, "wk", f"wk{l}")
                wv_s = wload(wvh[l].rearrange("(ko p) m -> p ko m", p=128),
                             4, C, "wv", f"wv{l}")
                pw_s = wload(pwh[l].rearrange("(ko p) m -> p ko m", p=128),
                             4, C, "pw", f"pw{l}")
                w1_s = wload(w1h[l].rearrange("(ko p) m -> p ko m", p=128),
                             4, DFF, "w1", f"w1{l}")
                w2_s = wload(w2h[l].rearrange("(ko p) m -> p ko m", p=128),
                             16, C, "w2", f"w2{l}")

                def vcol(j, l=l):
                    return vecs_s[:, l * 40 + j:l * 40 + j + 1]

                def layer_norm(r, gcol, bcol, out_tag, vcol=vcol):
                    """r: [128,4,TN] f32r -> normalized y in slot out_tag"""
                    r2 = apool.tile([128, 4, TN], F32R, tag="vt", name="r2sq")
                    sum_ps = ps_sm.tile([1, TN], F32, tag="sm", name="sum_ps")
                    sq_ps = ps_sm.tile([1, TN], F32, tag="sm", name="sq_ps")
                    for ch in range(4):
                        nc.scalar.square(r2[:, ch], r[:, ch])
                    for ch in range(4):
                        nc.tensor.matmul(sum_ps[:], ones_c[:], r[:, ch],
                                         start=(ch == 0), stop=(ch == 3))
                    for ch in range(4):
                        nc.tensor.matmul(sq_ps[:], ones_c[:], r2[:, ch],
                                         start=(ch == 0), stop=(ch == 3))
                    musb = apool.tile([1, TN], F32R, tag="musb", name="musb")
                    nc.scalar.mul(musb[:], sum_ps[:], 1.0 / C)
                    mu2 = apool.tile([1, TN], F32, tag="mu2", name="mu2")
                    nc.scalar.square(mu2[:], musb[:])
                    varb = apool.tile([1, TN], F32, tag="varb", name="varb")
                    nc.vector.scalar_tensor_tensor(varb[:], sq_ps[:], 1.0 / C,
                                                   mu2[:], ALU.mult,
                                                   ALU.subtract)
                    sd = apool.tile([1, TN], F32, tag="sd", name="sd")
                    nc.scalar.activation(sd[:], varb[:], AF.Sqrt, bias=eps_t[:])
                    isd = apool.tile([1, TN], F32R, tag="isd", name="isd")
                    with nc.allow_low_precision(reason="f32r stats fine"):
                        nc.vector.reciprocal(isd[:], sd[:])
                    mu_bc = ps_bc.tile([128, TN], F32, tag="bc", name="mu_bc")
                    nc.tensor.matmul(mu_bc[:], ones_r[:], musb[:],
                                     start=True, stop=True)
                    is_bc = ps_bc.tile([128, TN], F32, tag="bc", name="is_bc")
                    nc.tensor.matmul(is_bc[:], ones_r[:], isd[:],
                                     start=True, stop=True)
                    y = apool.tile([128, 4, TN], F32R, tag=out_tag,
                                   name=f"y_{out_tag}")
                    for ch in range(4):
                        nc.vector.tensor_tensor(y[:, ch], r[:, ch], mu_bc[:],
                                                ALU.subtract)
                        nc.vector.scalar_tensor_tensor(y[:, ch], y[:, ch],
                                                       vcol(gcol + ch),
                                                       is_bc[:], ALU.mult,
                                                       ALU.mult)
                        nc.vector.tensor_scalar(y[:, ch], y[:, ch],
                                                vcol(bcol + ch), None, ALU.add)
                    return y

                def tile_body(it):
                    toks = ds(it * TN, TN)
                    xt = apool.tile([128, 4, TN], F32R, tag="xt", name="xt")
                    for ch in range(4):
                        nc.sync.dma_start(xt[:, ch], xres[ch, :, toks])

                    # QKV
                    qT = apool.tile([128, 4, TN], F32R, tag="x1c", name="qT")
                    kT = apool.tile([128, 4, TN], F32R, tag="kT", name="kT")
                    vt = apool.tile([128, 4, C], F32R, tag="vt", name="vt")
                    for ch in range(4):
                        q_ps = ps_mm.tile([128, TN], F32, tag="mm", name="q_ps")
                        for ko in range(4):
                            nc.tensor.matmul(
                                q_ps[:], wq_s[:, ko, ch * 128:(ch + 1) * 128],
                                xt[:, ko], start=(ko == 0), stop=(ko == 3))
                        nc.scalar.copy(qT[:, ch], q_ps[:])
                        k_ps = ps_mm.tile([128, TN], F32, tag="mm", name="k_ps")
                        for ko in range(4):
                            nc.tensor.matmul(
                                k_ps[:], wk_s[:, ko, ch * 128:(ch + 1) * 128],
                                xt[:, ko], start=(ko == 0), stop=(ko == 3))
                        nc.scalar.copy(kT[:, ch], k_ps[:])
                        v_ps = ps_mm.tile([128, C], F32, tag="mm", name="v_ps")
                        for ko in range(4):
                            nc.tensor.matmul(
                                v_ps[:], xt[:, ko, ch * 128:(ch + 1) * 128],
                                wv_s[:, ko, :], start=(ko == 0), stop=(ko == 3))
                        nc.vector.tensor_copy(vt[:, ch], v_ps[:])

                    # attention: hp-outer score/softmax, then all att@v
                    for hp in range(4):
                        qsrc = qT[:, hp].rearrange("p (g b t) -> p g b t",
                                                   g=8, b=8, t=8)
                        qdst = qbd[:].rearrange("p g (b h t) -> p g b h t",
                                                b=8, h=2, t=8)
                        nc.vector.tensor_copy(qdst[0:64, :, :, 0, :],
                                              qsrc[0:64])
                        nc.vector.tensor_copy(qdst[64:128, :, :, 1, :],
                                              qsrc[64:128])
                        for g8 in range(8):
                            s_ps = ps_sm.tile([128, 64], F32, tag="sm",
                                              name="s_ps")
                            nc.tensor.matmul(
                                s_ps[:], qbd[:, g8, :],
                                kT[:, hp, g8 * 64:(g8 + 1) * 64],
                                start=True, stop=False)
                            nc.tensor.matmul(s_ps[:], mL_s[:], mR_s[:],
                                             start=False, stop=True)
                            atte = atpool.tile([128, 64], F32R, tag="atte",
                                               name="atte")
                            nc.scalar.activation(atte[:], s_ps[:], AF.Exp)
                            zt = atpool.tile([128, 1], F32, tag="zt",
                                             name="zt")
                            nc.vector.tensor_reduce(zt[:], atte[:], AX.X,
                                                    ALU.add)
                            rt = atpool.tile([128, 1], F32, tag="rt",
                                             name="rt")
                            nc.vector.reciprocal(rt[:], zt[:])
                            nc.vector.tensor_scalar_mul(atte[:], atte[:],
                                                        rt[:])
                            at_ps = ps_sm.tile([64, 128], F32R, tag="sm",
                                               name="at_ps")
                            nc.tensor.transpose(at_ps[:], atte[:], id_s[:])
                            d = rhs_av[:, g8 // 2].rearrange(
                                "p h (gg b t) -> p h gg b t",
                                gg=2, b=8, t=8)[(g8 % 2) * 64:
                                                (g8 % 2) * 64 + 64,
                                                hp * 2:(hp + 1) * 2,
                                                g8 % 2, :, :]
                            s2 = at_ps[:].rearrange("p (b h t) -> p h b t",
                                                    b=8, h=2, t=8)
                            nc.vector.tensor_copy(d, s2)
                    ot = apool.tile([128, 4, TN], F32R, tag="x1c", name="ot")
                    for g16 in range(4):
                        for h in range(8):
                            o_ps = ps_sm.tile([64, 128], F32, tag="sm",
                                              name="o_ps")
                            nc.tensor.matmul(
                                o_ps[:], vt[:, g16, h * 64:(h + 1) * 64],
                                rhs_av[:, g16, h, :], start=True, stop=True)
                            nc.scalar.copy(
                                ot[(h % 2) * 64:(h % 2) * 64 + 64, h // 2,
                                   g16 * 128:(g16 + 1) * 128], o_ps[:])

                    # proj + residual
                    r1 = apool.tile([128, 16, TN], F32R, tag="big1",
                                    name="r1")
                    for ch in range(4):
                        p_ps = ps_mm.tile([128, TN], F32, tag="mm", name="p_ps")
                        for ko in range(4):
                            nc.tensor.matmul(
                                p_ps[:], pw_s[:, ko, ch * 128:(ch + 1) * 128],
                                ot[:, ko], start=(ko == 0), stop=(ko == 3))
                        nc.vector.scalar_tensor_tensor(
                            r1[:, ch], p_ps[:], vcol(16 + ch), xt[:, ch],
                            ALU.add, ALU.add)
                    x1 = layer_norm(r1[:, 0:4], 0, 4, "x1c")

                    # FFN
                    hrl = apool.tile([128, 16, TN], F32R, tag="big1",
                                     name="hrl")
                    for dch in range(16):
                        h_ps = ps_mm.tile([128, TN], F32, tag="mm", name="h_ps")
                        for ko in range(4):
                            nc.tensor.matmul(
                                h_ps[:], w1_s[:, ko, dch * 128:(dch + 1) * 128],
                                x1[:, ko], start=(ko == 0), stop=(ko == 3))
                        nc.scalar.activation(hrl[:, dch], h_ps[:], AF.Relu,
                                             bias=vcol(24 + dch))
                    r2n = apool.tile([128, 4, TN], F32R, tag="xt", name="r2n")
                    for ch in range(4):
                        f_ps = ps_mm.tile([128, TN], F32, tag="mm", name="f_ps")
                        for ko in range(16):
                            nc.tensor.matmul(
                                f_ps[:], w2_s[:, ko, ch * 128:(ch + 1) * 128],
                                hrl[:, ko], start=(ko == 0), stop=(ko == 15))
                        nc.vector.scalar_tensor_tensor(
                            r2n[:, ch], f_ps[:], vcol(20 + ch), x1[:, ch],
                            ALU.add, ALU.add)
                    x2 = layer_norm(r2n, 8, 12, "kT")

                    if l < L - 1:
                        for ch in range(4):
                            nc.sync.dma_start(xres[ch, :, toks], x2[:, ch])
                    else:
                        lg_ps = ps_mm.tile([128, TN], F32, tag="mm",
                                           name="lg_ps")
                        for ko in range(4):
                            nc.tensor.matmul(lg_ps[:], lmw_s[:, ko, :],
                                             x2[:, ko],
                                             start=(ko == 0), stop=(ko == 3))
                        lgs = apool.tile([128, 4, TN], F32, tag="vt",
                                         name="lgs")
                        nc.vector.tensor_scalar(lgs[0:V, 0, :], lg_ps[0:V, :],
                                                lmb_s[0:V, :], None, ALU.add)
                        nc.sync.dma_start(logT[:, toks], lgs[0:V, 0, :])

                if STATIC:
                    for _it in range(NT):
                        tile_body(_it)
                else:
                    with tc.For_i(0, NT, 1, hint_engines=(PE,)) as it:
                        tile_body(it)
    nc.compile()
    return nc


def _prep_inputs(idx, tok_table, pos_table, wq, wk, wv, proj_w, proj_b,
                 w1, b1, w2, b2, ln1_g, ln1_b, ln2_g, ln2_b, lm_w, lm_b):
    f = np.float32
    scale = C ** (-0.5)
    wqh = np.ascontiguousarray(
        wq.transpose(0, 2, 1, 3).reshape(L, C, C) * scale).astype(f)
    wkh = np.ascontiguousarray(
        wk.transpose(0, 2, 1, 3).reshape(L, C, C)).astype(f)
    wvh = np.ascontiguousarray(
        wv.transpose(0, 2, 1, 3).reshape(L, C, C)).astype(f)
    pwh = np.ascontiguousarray(proj_w).astype(f)
    w1h = np.ascontiguousarray(w1).astype(f)
    w2h = np.ascontiguousarray(w2).astype(f)
    lmwh = np.zeros((C, 128), f)
    lmwh[:, :V] = lm_w
    embp = np.zeros((128, C), f)
    embp[:V] = tok_table
    embp[96:96 + T] = pos_table
    vecsh = np.zeros((128, L * 40), f)
    for l in range(L):
        for ch in range(4):
            sl = slice(ch * 128, (ch + 1) * 128)
            vecsh[:, l * 40 + 0 + ch] = ln1_g[l, sl]
            vecsh[:, l * 40 + 4 + ch] = ln1_b[l, sl]
            vecsh[:, l * 40 + 8 + ch] = ln2_g[l, sl]
            vecsh[:, l * 40 + 12 + ch] = ln2_b[l, sl]
            vecsh[:, l * 40 + 16 + ch] = proj_b[l, sl]
            vecsh[:, l * 40 + 20 + ch] = b2[l, sl]
        for dch in range(16):
            vecsh[:, l * 40 + 24 + dch] = b1[l, dch * 128:(dch + 1) * 128]
    lmbh = np.zeros((128, 1), f)
    lmbh[:V, 0] = lm_b
    iotav = np.broadcast_to(
        np.arange(128, dtype=f)[:, None], (128, TN)).copy()
    posoh = np.zeros((8, TN), f)
    for c in range(TN):
        posoh[c % 8, c] = 1.0
    mLm = np.zeros((128, 128), f)
    mRm = np.zeros((128, 64), f)
    mLm[0, :] = -BIG
    mRm[0, :] = 1.0
    for j in range(8):
        for sp in range(8):
            kk = 1 + j * 8 + sp
            for t in range(8):
                if t >= sp:
                    for h in range(2):
                        mLm[kk, j * 16 + h * 8 + t] = BIG
            mRm[kk, j * 8 + sp] = 1.0
    ident = np.eye(128, dtype=f)
    shared = dict(embp=embp, wqh=wqh, wkh=wkh, wvh=wvh, pwh=pwh, w1h=w1h,
                  w2h=w2h, lmwh=lmwh, vecsh=vecsh, lmbh=lmbh, iotav=iotav,
                  posoh=posoh, maskL=mLm, maskR=mRm, identh=ident)
    in_maps = []
    for c in range(NCORES):
        m = dict(shared)
        m["idxf"] = np.ascontiguousarray(
            idx[c * SEQ:(c + 1) * SEQ].reshape(-1)).astype(f)
        in_maps.append(m)
    return in_maps


def kernel(**inputs):
    from concourse.bass_utils import run_bass_kernel_spmd
    if "nc" not in _CACHE:
        _CACHE["nc"] = _build()
    nc = _CACHE["nc"]
    in_maps = _prep_inputs(**{k: np.asarray(v) for k, v in inputs.items()})
    res = run_bass_kernel_spmd(nc, in_maps, list(range(NCORES)))
    out = np.empty((B, T, V), np.float32)
    for c in range(NCORES):
        lt = res.results[c]["logT"]           # [V, TOK]
        out[c * SEQ:(c + 1) * SEQ] = (
            lt.reshape(V, SEQ, T).transpose(1, 2, 0))
    return out


if __name__ == "__main__":
    _build()
    print("built ok")
